# revision 2
# baseline (speedup 1.0000x reference)
"""v5 Trainium2 Bass kernel.

v3 scheme (8 cores x 8 topo batches: rdma all-gather -> layernorm ->
topo attention -> per-neuron attention (TL=16) -> masked affine ->
adaptive gelu -> rdma handoff) plus:

  - Query packing: mask is a host-known input and zeroes ~half the
    neuron-attention queries (w*mask kills their affine contribution).
    Only the P=max-alive (84) query columns per neuron are computed:
    scores/exp/PV shrink 128->84 cols per tl, laid out bank-aligned
    ([S,4P] per PSUM bank) so every matmul/exp slice stays in-bank.
  - u/Delta split: q/k/v projections contract u' = u + topo-delta, so
    the (packed) Q-broadcast + bias matmuls run on u DURING the topo
    attention and only a small Delta-matmul per bank remains on the
    critical path before the exps.
  - Wq, Wk, q-bias, broadcast stationaries in fp8e3 (e3m4); weight
    streams split across the SP and ACT DMA queues.
  - ada gain folded into the sel placement matrix (host); tanh bias on
    the ACT bias port; single [1,2] stats matmul; -mu folding.
"""
import sys
import numpy as np

sys.path.insert(0, "/opt/trn_rl_repo")

I, L, T, S = 128, 8, 128, 128
N_CORES = 8
TL = T // N_CORES
EPS = 1e-5
RS = float(1.0 / np.sqrt(np.float32(S)))
GC = 0.7978845608028654
GA = 0.044715
MAGIC = 0x5F3759DF

_cached = None
_cached_P = None


def _patch_topology():
    """No /dev/neuron* client-side: give the sim the static TRN2 NC map it
    needs to route remote DMA (the NEFF itself uses relative XOR routing)."""
    from concourse import libnrt
    base = (0, 1, 2, 3, 6, 7, 4, 5)

    def get_trn2_nc_mapping():
        return {(d, k): base[k] for d in range(16) for k in range(8)}

    def nc_to_real_nc(device_index, nc_index):
        return base[nc_index]

    def pnc_id_to_device_and_real_nc_index(core_id):
        return core_id // 8, base[core_id % 8]

    def get_device_id_to_routing_id_mapping():
        return {d: d for d in range(16)}

    libnrt.get_trn2_nc_mapping = get_trn2_nc_mapping
    libnrt.nc_to_real_nc = nc_to_real_nc
    libnrt.pnc_id_to_device_and_real_nc_index = pnc_id_to_device_and_real_nc_index
    libnrt.get_device_id_to_routing_id_mapping = get_device_id_to_routing_id_mapping
    for modname in ("concourse.bass_interp", "concourse.dge_state"):
        m = sys.modules.get(modname)
        if m is None:
            continue
        for fn in (nc_to_real_nc, pnc_id_to_device_and_real_nc_index,
                   get_device_id_to_routing_id_mapping):
            if hasattr(m, fn.__name__):
                setattr(m, fn.__name__, fn)


def _build(P):
    _patch_topology()
    from concourse import bacc, tile, mybir

    fp32 = mybir.dt.float32
    bf16 = mybir.dt.bfloat16
    f8 = mybir.dt.float8e3
    int8 = mybir.dt.int8
    int32 = mybir.dt.int32
    Exp = mybir.ActivationFunctionType.Exp
    Tanh = mybir.ActivationFunctionType.Tanh
    mul_op = mybir.AluOpType.mult
    add_op = mybir.AluOpType.add
    sub_op = mybir.AluOpType.subtract
    shr_op = mybir.AluOpType.arith_shift_right
    AxX = mybir.AxisListType.X

    QC = 16 * P            # packed query columns total
    P4 = 4 * P             # per PSUM bank

    nc = bacc.Bacc("TRN2", target_bir_lowering=False, debug=False,
                   enable_asserts=True, num_devices=N_CORES)

    qk8_d = nc.dram_tensor("qk8", [L, S, QC + 2048], int8, kind="ExternalInput").ap()
    vcb_d = nc.dram_tensor("vcb", [L, S, 2048], bf16, kind="ExternalInput").ap()
    top_d = nc.dram_tensor("top", [L, S, 432], bf16, kind="ExternalInput").ap()
    bqr_d = nc.dram_tensor("bqr", [L, QC], int8, kind="ExternalInput").ap()
    sg_d = nc.dram_tensor("sg", [L, TL, S], fp32, kind="ExternalInput").ap()
    trow_d = nc.dram_tensor("trow", [1, L * 2 * S], bf16, kind="ExternalInput").ap()
    pre_d = nc.dram_tensor("pre", [S, 322], fp32, kind="ExternalInput").ap()
    wbr_d = nc.dram_tensor("wbr", [1, L * TL], fp32, kind="ExternalInput").ap()
    thr_d = nc.dram_tensor("thr", [1, 8], int32, kind="ExternalInput").ap()
    magic_d = nc.dram_tensor("magic", [1, 2], int32, kind="ExternalInput").ap()
    out_d = nc.dram_tensor("out", [TL, 1], fp32, kind="ExternalOutput").ap()

    rsems = [nc.alloc_semaphore(f"rsem{b}") for b in range(L - 1)]
    lsem = nc.alloc_semaphore("lsem")

    with tile.TileContext(nc) as tc:
        with tc.tile_pool(name="wpool", bufs=3) as wpool, \
             tc.tile_pool(name="vpool", bufs=3) as vpool, \
             tc.tile_pool(name="spool", bufs=3) as spool, \
             tc.tile_pool(name="fixed", bufs=1) as fixed, \
             tc.tile_pool(name="work", bufs=1) as work, \
             tc.tile_pool(name="ps_big", bufs=1, space="PSUM") as ps_big, \
             tc.tile_pool(name="ps_sm", bufs=1, space="PSUM") as ps_sm, \
             tc.tile_pool(name="ps_kv", bufs=1, space="PSUM") as ps_kv, \
             tc.tile_pool(name="ps_tp", bufs=1, space="PSUM") as ps_tp:

            pre = fixed.tile([S, 322], fp32)
            nc.sync.dma_start(pre[:], pre_d)
            magic = fixed.tile([1, 2], int32)
            nc.scalar.dma_start(magic[:], magic_d)
            trow = fixed.tile([1, L * 2 * S], bf16)
            nc.scalar.dma_start(trow[:], trow_d)
            thr = fixed.tile([1, 8], int32)
            wbr = fixed.tile([1, L * TL], fp32)
            ones_col = fixed.tile([S, 1], fp32)
            nc.vector.memset(ones_col[:], 1.0)
            ones_row = fixed.tile([1, S], fp32)
            nc.vector.memset(ones_row[:], 1.0)
            ones_row_f8 = fixed.tile([1, S], f8)
            nc.vector.memset(ones_row_f8[:], 1.0)
            ones_row_bf = fixed.tile([1, S], bf16)
            nc.vector.memset(ones_row_bf[:], 1.0)
            ones_mat_f8 = fixed.tile([S, S], f8)
            nc.vector.memset(ones_mat_f8[:], 1.0)
            one_one = fixed.tile([1, 1], fp32)
            nc.vector.memset(one_one[:], 1.0)
            pvr_t = fixed.tile([S, 2], bf16)
            nc.vector.memset(pvr_t[:], 1.0)
            vv = fixed.tile([S, 2], fp32)          # [ones | v]
            nc.vector.memset(vv[:, 0:1], 1.0)

            bsrc = [fixed.tile([S, 1], fp32, name=f"bsrc{b}") for b in range(L - 1)]
            v8s = [fixed.tile([S, 7], fp32, name=f"v8_{b}") for b in range(L - 1)]

            v_bf = work.tile([S, 1], bf16)
            u_col = work.tile([S, 1], fp32)
            u_bf = work.tile([S, 1], bf16)
            d_col = work.tile([S, 1], fp32)
            d_bf = work.tile([S, 1], bf16)
            up_col = work.tile([S, 1], fp32)
            Ub8 = work.tile([S, S], f8)
            Db8 = work.tile([S, S], f8)
            sc = work.tile([1, 12], fp32)
            sci = sc[:].bitcast(int32)
            bq_row = work.tile([1, S], fp32)
            qh_row = work.tile([1, S], bf16)
            cmv2 = work.tile([S, 1], fp32)
            qkvt_v = work.tile([S, 1], fp32)
            rstd_sb = work.tile([S, 1], fp32)
            rsb2 = work.tile([S, 2], fp32)
            kbrs = work.tile([S, 1], fp32)
            khrs = work.tile([S, 1], fp32)
            Et_sb = work.tile([S, S], bf16)
            v_tmp = work.tile([S, 1], fp32)
            krsA = work.tile([S, TL // 2], fp32)
            krsB = work.tile([S, TL // 2], fp32)
            pvr_n = work.tile([S, 2 * TL], bf16)
            sct = [work.tile([S, P4], bf16, name=f"sct{h}") for h in range(2)]
            eat = [work.tile([S, P4], bf16, name=f"eat{h}") for h in range(2)]
            ebt = [work.tile([S, P], bf16, name=f"ebt{j}") for j in range(8)]
            rden = work.tile([S, TL], fp32)
            zp = work.tile([S, TL], fp32)
            afr = work.tile([S, 2 * TL], fp32)
            aff_sb = work.tile([TL, 1], fp32)
            gcx = work.tile([S, 1], fp32)
            xgh = work.tile([S, 1], fp32)
            s2t = work.tile([S, 1], fp32)
            t1a = work.tile([S, 1], fp32)
            t1t = work.tile([S, 1], fp32)

            scoresA = ps_big.tile([S, 1024], fp32)       # banks for tls 0-7
            scoresB = ps_big.tile([S, 1024], fp32)       # banks for tls 8-15
            smps = ps_sm.tile([S, 512], fp32)            # 1 bank
            af_ps = smps[0:TL, 64:65]
            v128_ps = smps[:, 66:67]
            sv2_ps = smps[0:1, 68:70]
            bc_ps = smps[:, 70:74]
            k0_ps = smps[:, 74:75]
            v0_ps = smps[:, 76:77]
            pvt_ps = smps[:, 78:80]
            kvps_t = ps_kv.tile([S, 512], fp32)          # 1 bank
            kv_ps = kvps_t[:, 0:32]
            pvn_ps = kvps_t[:, 32:64]
            tpps = ps_tp.tile([S, 512], fp32)            # 1 bank (topo stage)
            q0_ps = tpps[0:1, 128:256]
            tsc_ps = tpps[:, 0:128]

            rthr_cm = nc.vector.register("rthr")
            rthr = rthr_cm.__enter__()

            def ts(out, in0, s1, op0, s2=None, op1=None, eng=None):
                e = eng or nc.vector
                if s2 is None:
                    e.tensor_scalar(out, in0, s1, None, op0)
                else:
                    e.tensor_scalar(out, in0, s1, s2, op0, op1)

            for b in range(L):
                # ---- weight prefetch ----
                qk8i = wpool.tile([S, QC + 2048], int8, tag="qk8")
                vcb = vpool.tile([S, 2048], bf16, tag="vcb")
                top = spool.tile([S, 432], bf16, tag="top")
                bqri = spool.tile([1, QC], int8, tag="bqr")
                nc.sync.dma_start(qk8i[:], qk8_d[b])
                nc.sync.dma_start(vcb[:], vcb_d[b])
                nc.sync.dma_start(bqri[:], bqr_d[b])
                qk8 = qk8i[:].bitcast(f8)
                bqr = bqri[:].bitcast(f8)
                nc.sync.dma_start(top[:], top_d[b])
                if b < L - 1:
                    sg = spool.tile([TL, S], fp32, tag="sg")
                    nc.sync.dma_start(sg[:], sg_d[b])
                if b == 0:
                    nc.sync.dma_start(thr[:], thr_d)
                    nc.scalar.dma_start(wbr[:], wbr_d)
                    # Tracked WAW edge: reg_save writes a byte of v_tmp, so
                    # every later v_tmp writer (incl. the sem-waiting reduce)
                    # orders after the reg_load (register deps inside wait
                    # conditions are not tracked by tile).
                    nc.vector.reg_load(rthr, thr[0:1, 0:1])
                    nc.vector.reg_save(v_tmp[0:1, 0:1].bitcast(int32), rthr)
                    nc.vector.reg_load(rthr, thr[0:1, 0:1])
                    nc.vector.reg_save(v_tmp[0:1, 0:1].bitcast(int32), rthr)
                bkrs = top[:, 384:400]
                bvm = top[:, 400:416]
                mt = top[:, 416:432]
                wmtP = pre[0:P, 18 + 16 * b:18 + 16 * (b + 1)]
                wmtF = pre[:, 194 + 16 * b:194 + 16 * (b + 1)]
                gam = pre[:, 146 + b:147 + b]
                bet = pre[:, 154 + b:155 + b]
                g1h = pre[:, 10 + b:11 + b]
                tck = pre[:, 178 + b:179 + b]
                bpkrs = pre[:, 186 + b:187 + b]

                # ---- desc-gen for THIS batch's end-of-batch broadcast ----
                if b < L - 1:
                    if b >= 2:
                        # SWDGE ring holds ~14 preps. Dummy write to bsrc[b]
                        # reading bsrc[b-2] (a declared output of trigger
                        # b-2): the preps' no-sync src edge then orders them
                        # after trigger b-2 on the in-order Pool queue, so
                        # ring entries are reclaimed before desc-gen.
                        ts(bsrc[b][0:1, 0:1], ones_row[0:1, 0:1],
                           bsrc[b - 2][0:1, 0:1], mul_op)
                    for k in range(1, N_CORES):
                        rd = [None] * 8
                        rd[k] = (0, k)
                        nc.gpsimd.remote_dma_broadcast(
                            v8s[b][:, k - 1:k], bsrc[b][:],
                            rsems[b], lsem, rdests=rd)

                # ---- acquire v ----
                if b == 0:
                    nc.vector.tensor_copy(vv[:, 1:2], pre[:, 0:1])
                else:
                    red = nc.vector.tensor_reduce(v_tmp[:], v8s[b - 1][:],
                                                  AxX, add_op)
                    red.wait_op(rsems[b - 1], rthr, "sem-ge")
                    nc.vector.tensor_add(vv[:, 1:2], v_tmp[:], bsrc[b - 1][:])
                v_col = vv[:, 1:2]

                # ---- topo qkv on raw v (PE, parallel with stats): q as a
                # row (for the broadcast matmul), k and v as columns ----
                nc.vector.tensor_copy(v_bf[:], v_col)
                nc.tensor.matmul(q0_ps, v_bf[:], top[:, 0:S],
                                 start=True, stop=True)
                nc.tensor.matmul(k0_ps, top[:, S:2 * S], v_bf[:],
                                 start=True, stop=True)
                nc.tensor.matmul(v0_ps, top[:, 2 * S:3 * S], v_bf[:],
                                 start=True, stop=True)

                # ---- stats (one matmul) + Newton-1 rsqrt; sc0 = -mu ----
                nc.tensor.matmul(sv2_ps, v_col, vv[:, 0:2], start=True, stop=True)
                ts(sc[:, 0:1], sv2_ps[0:1, 0:1], -1.0 / S, mul_op)
                ts(sc[:, 1:2], sv2_ps[0:1, 1:2], 1.0 / S, mul_op)
                nc.vector.scalar_tensor_tensor(sc[:, 3:4], sc[:, 0:1], sc[:, 0:1],
                                               sc[:, 1:2], mul_op, sub_op)
                ts(sc[:, 4:5], sc[:, 3:4], -1.0, mul_op, EPS, add_op)      # vpe
                ts(sc[:, 5:6], sc[:, 3:4], -0.5, mul_op, 0.5 * EPS, add_op)  # vh
                ts(sci[:, 8:9], sci[:, 4:5], 1, shr_op)
                nc.vector.tensor_sub(sci[:, 6:7], magic[:, 0:1], sci[:, 8:9])
                nc.vector.scalar_tensor_tensor(sc[:, 8:9], sc[:, 6:7], sc[:, 5:6],
                                               sc[:, 6:7], mul_op, mul_op)
                ts(sc[:, 8:9], sc[:, 8:9], -1.0, mul_op, 1.5, add_op)
                nc.vector.tensor_mul(sc[:, 6:7], sc[:, 6:7], sc[:, 8:9])   # rstd
                nc.vector.tensor_mul(sc[:, 7:8], sc[:, 6:7], sc[:, 0:1])   # -mu*rstd
                ts(sc[:, 9:10], sc[:, 6:7], RS, mul_op)                    # rstd*RS
                ts(sc[:, 10:11], sc[:, 7:8], RS, mul_op)                   # -mu*rstd*RS
                nc.tensor.matmul(bc_ps[:, 0:2], ones_row[:], sc[:, 6:8],
                                 start=True, stop=True, skip_group_check=True)
                nc.tensor.matmul(bc_ps[:, 2:4], ones_row[:], sc[:, 9:11],
                                 start=True, stop=True, skip_group_check=True)
                rstd_c = bc_ps[:, 0:1]
                nmur_c = bc_ps[:, 1:2]

                # ---- u = rstd*gamma*(v-mu) + beta  (scalars from PSUM) ----
                grstd = work.tile([S, 1], fp32, tag="grstd")
                boff = work.tile([S, 1], fp32, tag="boff")
                ts(grstd[:], gam, rstd_c, mul_op)
                nc.vector.scalar_tensor_tensor(boff[:], gam, nmur_c, bet,
                                               mul_op, add_op)
                nc.vector.scalar_tensor_tensor(u_col[:], v_col,
                                               grstd[:, 0:1], boff[:, 0:1],
                                               mul_op, add_op)
                nc.gpsimd.tensor_copy(u_bf[:], u_col[:])
                ts(Ub8[:], ones_mat_f8[:], u_col[:, 0:1], mul_op,
                   eng=nc.gpsimd)
                u8_col = Ub8[:, 0:1]

                # ---- Q bias + Q@u for all 4 banks, k/v@u columns: these run
                # during the topo attention (only need u) ----
                for half, dst in ((0, scoresA), (1, scoresB)):
                    for hb in range(2):
                        t0 = (2 * half + hb) * 4
                        ds = slice(hb * 512, hb * 512 + P4)
                        cs = slice(t0 * P, t0 * P + P4)
                        nc.tensor.matmul(dst[:, ds], ones_row_f8[:],
                                         bqr[:, cs], start=True, stop=False,
                                         skip_group_check=True)
                        nc.tensor.matmul(dst[:, ds], Ub8[:], qk8[:, cs],
                                         start=False, stop=False,
                                         skip_group_check=True)
                # One start=True for the whole ps_kv bank: a start re-marks
                # the full 2KB zero-region pending, so a second start here
                # would wipe the accumulated u-part before the Delta pass.
                for tl in range(TL):
                    nc.tensor.matmul(kv_ps[:, 2 * tl:2 * tl + 1],
                                     qk8[:, QC + tl * S:QC + (tl + 1) * S],
                                     u8_col, start=(tl == 0), stop=False,
                                     skip_group_check=True)
                    nc.tensor.matmul(kv_ps[:, 2 * tl + 1:2 * tl + 2],
                                     vcb[:, tl * S:(tl + 1) * S],
                                     u_bf[:], start=False, stop=False,
                                     skip_group_check=True)

                # ---- topo attention: q-hat row (2 stt) -> broadcast matmul;
                # k-hat*rs as a column feeding the exp's per-partition scale ----
                tcq_row = trow[0:1, b * 2 * S:b * 2 * S + S]
                bpq_row = trow[0:1, b * 2 * S + S:(b + 1) * 2 * S]
                nc.vector.scalar_tensor_tensor(bq_row[:], tcq_row, sc[:, 7:8],
                                               bpq_row, mul_op, add_op)
                nc.vector.scalar_tensor_tensor(qh_row[:], q0_ps, sc[:, 6:7],
                                               bq_row[:], mul_op, add_op)
                nc.tensor.matmul(tsc_ps, ones_row_bf[:], qh_row[:],
                                 start=True, stop=True)
                nc.vector.tensor_copy(rsb2[:], bc_ps[:, 2:4])
                nc.vector.scalar_tensor_tensor(kbrs[:], tck, rsb2[:, 1:2],
                                               bpkrs, mul_op, add_op)
                nc.vector.scalar_tensor_tensor(khrs[:], k0_ps, rsb2[:, 0:1],
                                               kbrs[:], mul_op, add_op)
                nc.vector.scalar_tensor_tensor(cmv2[:], pre[:, 162 + b:163 + b],
                                               nmur_c, pre[:, 170 + b:171 + b],
                                               mul_op, add_op)
                nc.vector.tensor_copy(rstd_sb[:], rstd_c)
                nc.vector.scalar_tensor_tensor(qkvt_v[:], v0_ps,
                                               rstd_sb[:, 0:1], cmv2[:],
                                               mul_op, add_op)
                nc.scalar.activation(Et_sb[:], tsc_ps, Exp, scale=khrs[:, 0:1])
                nc.gpsimd.tensor_copy(pvr_t[:, 0:1], qkvt_v[:])
                nc.tensor.matmul(pvt_ps, Et_sb[:], pvr_t[:], start=True, stop=True)
                rd1 = work.tile([S, 1], fp32, tag="rd1")
                nc.vector.reciprocal(rd1[:], pvt_ps[:, 1:2])
                ts(d_col[:], pvt_ps[:, 0:1], rd1[:, 0:1], mul_op)  # topo delta
                nc.gpsimd.tensor_add(up_col[:], d_col[:], u_col[:])
                nc.vector.tensor_copy(d_bf[:], d_col[:])
                ts(Db8[:], ones_mat_f8[:], d_col[:, 0:1], mul_op)
                d8_col = Db8[:, 0:1]

                # ---- Delta accumulation: finish kv columns, then per-bank
                # Q@Delta; B banks first so the fused ACT exps start earliest ----
                for tl in range(TL):
                    nc.tensor.matmul(kv_ps[:, 2 * tl:2 * tl + 1],
                                     qk8[:, QC + tl * S:QC + (tl + 1) * S],
                                     d8_col, start=False, stop=True,
                                     skip_group_check=True)
                    nc.tensor.matmul(kv_ps[:, 2 * tl + 1:2 * tl + 2],
                                     vcb[:, tl * S:(tl + 1) * S],
                                     d_bf[:], start=False, stop=True,
                                     skip_group_check=True)
                kv2 = kv_ps.rearrange("p (t k) -> p t k", k=2)
                nc.vector.scalar_tensor_tensor(krsB[:], kv2[:, 8:16, 0], RS,
                                               bkrs[:, 8:16], mul_op, add_op)
                nc.vector.scalar_tensor_tensor(krsA[:], kv2[:, 0:8, 0], RS,
                                               bkrs[:, 0:8], mul_op, add_op)
                p2 = pvr_n[:].rearrange("p (t k) -> p t k", k=2)
                nc.vector.scalar_tensor_tensor(p2[:, :, 0], kv2[:, :, 1], 1.0,
                                               mt, mul_op, mul_op)
                nc.gpsimd.tensor_add(p2[:, :, 0], p2[:, :, 0], bvm)
                nc.gpsimd.tensor_copy(p2[:, :, 1], mt)

                for half, dst in ((1, scoresB), (0, scoresA)):
                    for hb in range(2):
                        t0 = (2 * half + hb) * 4
                        ds = slice(hb * 512, hb * 512 + P4)
                        cs = slice(t0 * P, t0 * P + P4)
                        nc.tensor.matmul(dst[:, ds], Db8[:], qk8[:, cs],
                                         start=False, stop=True,
                                         skip_group_check=True)

                # ---- k*rs scale + exp: fused-ACT narrow exps for tls 8-15
                # (B banks, first), DVE-scale + wide ACT exp for tls 0-7.
                # Per-block E tiles keep tile-granular edges from chaining
                # exp(j) behind PV(j-1). ----
                pv2 = pvn_ps[0:P, :].rearrange("p (t k) -> p t k", k=2)
                for hb in range(2):
                    for j in range(4):
                        tl = 8 + 4 * hb + j
                        eb = ebt[4 * hb + j]
                        nc.scalar.activation(eb[:], scoresB[:, hb * 512 + j * P:
                                                            hb * 512 + (j + 1) * P],
                                             Exp, scale=krsB[:, 4 * hb + j:
                                                            4 * hb + j + 1])
                        nc.tensor.matmul(pvn_ps[0:P, 2 * tl:2 * tl + 2],
                                         eb[:], pvr_n[:, 2 * tl:2 * tl + 2],
                                         start=True, stop=True)
                for hb in range(2):
                    for j in range(4):
                        tl = 4 * hb + j
                        ts(sct[hb][:, j * P:(j + 1) * P],
                           scoresA[:, hb * 512 + j * P:hb * 512 + (j + 1) * P],
                           krsA[:, tl:tl + 1], mul_op)
                    nc.scalar.activation(eat[hb][:], sct[hb][:], Exp)
                    for j in range(4):
                        tl = 4 * hb + j
                        nc.tensor.matmul(pvn_ps[0:P, 2 * tl:2 * tl + 2],
                                         eat[hb][:, j * P:(j + 1) * P],
                                         pvr_n[:, 2 * tl:2 * tl + 2],
                                         start=True, stop=True)

                # ---- aff = sum_i wmt*(zp + u') + wbias; B half first so its
                # divide runs under the A exps ----
                nc.vector.reciprocal(rden[0:P, 8:16], pv2[:, 8:16, 1])
                nc.vector.tensor_mul(zp[0:P, 8:16], pv2[:, 8:16, 0],
                                     rden[0:P, 8:16])
                nc.vector.tensor_mul(afr[0:P, 8:16], wmtP[:, 8:16],
                                     zp[0:P, 8:16])
                nc.vector.reciprocal(rden[0:P, 0:8], pv2[:, 0:8, 1])
                nc.vector.tensor_mul(zp[0:P, 0:8], pv2[:, 0:8, 0],
                                     rden[0:P, 0:8])
                nc.vector.tensor_mul(afr[0:P, 0:8], wmtP[:, 0:8],
                                     zp[0:P, 0:8])
                nc.tensor.matmul(af_ps, afr[0:P, 0:TL], ones_col[0:P, :],
                                 start=True, stop=False, skip_group_check=True)
                nc.tensor.matmul(af_ps, wmtF, up_col[:], start=False,
                                 stop=False, skip_group_check=True)
                nc.tensor.matmul(af_ps, wbr[:, b * TL:(b + 1) * TL], one_one[:],
                                 start=False, stop=True, skip_group_check=True)

                if b == L - 1:
                    nc.vector.tensor_copy(aff_sb[:], af_ps)
                    nc.sync.dma_start(out_d, aff_sb[:])
                else:
                    nc.vector.tensor_copy(aff_sb[:], af_ps)
                    # xg lands sel-placed with ada gain pre-folded into sg
                    nc.tensor.matmul(v128_ps, sg[:], aff_sb[:],
                                     start=True, stop=True)
                    # adaptive gelu (tanh approx): tanh(GC*xg + GC*GA*xg^3)
                    ts(gcx[:], v128_ps, GC, mul_op)
                    ts(xgh[:], v128_ps, g1h[:, 0:1], mul_op)
                    nc.vector.tensor_mul(s2t[:], gcx[:], gcx[:])
                    nc.vector.scalar_tensor_tensor(t1a[:], s2t[:], GA / (GC * GC),
                                                   v128_ps, mul_op, mul_op)
                    nc.scalar.activation(t1t[:], t1a[:], Tanh, scale=GC,
                                         bias=gcx[:, 0:1])
                    # WAW anchor for the next batch's reduce: without it the
                    # scheduler hoists the (sem-blocked) reduce to the head
                    # of the in-order DVE queue and wedges the whole engine.
                    ts(v_tmp[0:1, 0:1], ones_row[0:1, 0:1],
                       t1t[0:1, 0:1], mul_op)
                    nc.vector.scalar_tensor_tensor(bsrc[b][:], t1t[:], 1.0,
                                                   xgh[:], add_op, mul_op)
                    nc.gpsimd.trigger_dma(count=None,
                                          signals_writable=[bsrc[b][:]])

    nc.compile()
    return nc


def _host_prep(x, W, mask, attn_t, attn_n, norm_params, ada):
    import ml_dtypes
    f32 = np.float32
    bf16 = ml_dtypes.bfloat16
    f8 = ml_dtypes.float8_e3m4
    x, W, mask, attn_t, attn_n, norm_params, ada = (
        np.ascontiguousarray(np.asarray(a, f32))
        for a in (x, W, mask, attn_t, attn_n, norm_params, ada))
    gamma = norm_params[:, 0, :]
    beta = norm_params[:, 1, :]

    P = int(mask.sum(2).max())
    P = max(P, 8)

    topo_w = attn_t[:, :, :, :S]
    topo_b = attn_t[:, :, :, S]
    topo_wg = topo_w * gamma[:, None, None, :]
    topo_wt_flat = np.ascontiguousarray(
        topo_wg.transpose(0, 3, 1, 2)).reshape(L, S, 3 * S)
    topo_c = topo_wg.sum(axis=3)
    topo_bp = np.einsum('lmis,ls->lmi', topo_w, beta) + topo_b

    wmat = W[:, :, :S] * mask
    wbias = W[:, :, S]

    pre0 = np.zeros((S, 322), f32)
    pre0[:, 0] = x
    pre0[:, 2:10] = ada[:, :, 0].T
    pre0[:, 10:18] = (0.5 * ada[:, :, 1]).astype(f32).T
    pre0[:, 146:154] = gamma.T
    pre0[:, 154:162] = beta.T
    pre0[:, 162:170] = topo_c[:, 2, :].T
    pre0[:, 170:178] = topo_bp[:, 2, :].T
    pre0[:, 178:186] = topo_c[:, 1, :].T                  # tck
    pre0[:, 186:194] = (RS * topo_bp[:, 1, :]).astype(f32).T  # bpk*rs

    magic = np.array([[MAGIC, 0]], np.int32)
    thr = np.full((1, 8), 14, np.int32)

    # packed query index map per (batch, global neuron): alive first, pad 0
    packs = np.zeros((L, T, P), np.int64)
    npacks = np.zeros((L, T), np.int64)
    for b in range(L):
        for t in range(T):
            idx = np.nonzero(mask[b, t] > 0)[0]
            packs[b, t, :len(idx)] = idx
            npacks[b, t] = len(idx)

    in_maps = []
    for c in range(N_CORES):
        sl = slice(c * TL, (c + 1) * TL)
        an = attn_n[:, sl]
        anw = an[:, :, :, :, :S]                              # (L,TL,3,i,p)
        anb = an[:, :, :, :, S]                               # (L,TL,3,i)
        # packed q weights/bias: (L, S, TL*P) col = tl*P + r
        qp = np.zeros((L, S, TL * P), f32)
        bq = np.zeros((L, TL * P), f32)
        wmtp = np.zeros((L, P, TL), f32)
        for b in range(L):
            for tl in range(TL):
                t = c * TL + tl
                n = npacks[b, t]
                idx = packs[b, t, :n]
                qp[b, :, tl * P:tl * P + n] = anw[b, tl, 0, idx, :].T
                bq[b, tl * P:tl * P + n] = anb[b, tl, 0, idx]
                wmtp[b, :n, tl] = wmat[b, t, idx]
        kpart = np.ascontiguousarray(
            anw[:, :, 1].transpose(0, 3, 1, 2)).reshape(L, S, TL * S)
        vpart = np.ascontiguousarray(
            anw[:, :, 2].transpose(0, 3, 1, 2)).reshape(L, S, TL * S)
        qk8 = np.concatenate(
            [qp.astype(f8).view(np.int8), kpart.astype(f8).view(np.int8)],
            axis=2)
        vcb = vpart.astype(bf16)
        mtt = mask[:, sl].transpose(0, 2, 1)                     # (L, i, TL)
        bkrs_t = (RS * anb[:, :, 1, :]).transpose(0, 2, 1)       # (L, i, TL)
        bvm_t = (anb[:, :, 2, :].transpose(0, 2, 1) * mtt)       # bv*m
        top = np.concatenate([topo_wt_flat, bkrs_t, bvm_t, mtt],
                             axis=2).astype(bf16)
        bqr = bq.astype(f8).view(np.int8)
        premap = pre0.copy()
        premap[:, 18:146] = 0.0
        premap[:P, 18:146] = wmtp.transpose(1, 0, 2).reshape(P, L * TL)
        premap[:, 194:322] = wmat[:, sl].transpose(0, 2, 1).transpose(
            1, 0, 2).reshape(S, L * TL)
        trow = np.ascontiguousarray(np.concatenate(
            [topo_c[:, 0, :], topo_bp[:, 0, :]],
            axis=1).reshape(1, L * 2 * S)).astype(bf16)
        sg = np.zeros((L, TL, S), f32)
        for b in range(L):
            for j in range(TL):
                sg[b, j, c * TL + j] = ada[b, c * TL + j, 0]
        wbr = np.ascontiguousarray(wbias[:, sl].reshape(1, L * TL))
        in_maps.append(dict(qk8=qk8, vcb=vcb, top=top, bqr=bqr, sg=sg,
                            trow=trow, pre=premap, wbr=wbr,
                            thr=thr, magic=magic))
    return P, in_maps


def kernel(x, W, mask, attn_t, attn_n, attn_mask_n, norm_params, ada,
           span_ids, tb_ids):
    global _cached, _cached_P
    _patch_topology()
    from concourse import bass_utils
    P, in_maps = _host_prep(x, W, mask, attn_t, attn_n, norm_params, ada)
    if _cached is None or _cached_P != P:
        _cached = _build(P)
        _cached_P = P
    nc = _cached
    res = bass_utils.run_bass_kernel_spmd(nc, in_maps, core_ids=list(range(N_CORES)))
    out = np.concatenate([res.results[c]["out"].reshape(TL) for c in range(N_CORES)])
    return out.astype(np.float32)
